# revision 1
# baseline (speedup 1.0000x reference)
"""Causal MHA (B=2, S=2048, D=2048, H=16) on 8 trn2 NeuronCores.

Sharding: tensor-parallel over heads. Each core computes QKV + RoPE + causal
SDPA for H/8 heads end-to-end, then an AllToAll redistributes attention
outputs from head-sharded to token-sharded layout, and each core computes the
full out-projection for its 1/8 token slice (avoids the 32MB AllReduce the
naive row-parallel scheme would need; A2A moves only 4MB/core).

Device layouts (everything feature-major, partition dim = 128):
  xT      [D, T]        x transposed (T = B*S tokens, b-major)
  q/k/v   [128, HPC*T]  per-core; q/k feature rows are per-head permuted to
                        [even(64); odd(64)] so RoPE's pair rotation becomes a
                        64-partition block swap (done via 2 small DMAs)
  scores  S^T tiles [tk=128, tq=512]: softmax denominator comes from an
                        all-ones lhsT matmul accumulated alongside P@V
  out-proj: out[t, j] token-major so the host just concatenates core blocks.

All matmuls use float32r (full-rate fp32 on the PE at N>=256).
"""

import numpy as np

import concourse.bass as bass
import concourse.bacc as bacc
import concourse.mybir as mybir
import concourse.tile as tile
from concourse import bass_utils

F32 = mybir.dt.float32
F32R = mybir.dt.float32r
NEG = -1e9


class Cfg:
    def __init__(self, B, S, D, H, NC=8):
        self.B, self.S, self.D, self.H, self.NC = B, S, D, H, NC
        self.DK = D // H
        assert self.DK == 128, "kernel assumes head dim 128"
        self.T = B * S                 # tokens, b-major
        self.HPC = H // NC             # heads per core
        self.FPC = self.HPC * self.DK  # features per core (q or k or v)
        self.W3 = 3 * self.FPC
        self.DCH = D // 128            # contraction chunks
        self.TT = 512                  # qkv token tile
        self.NTT = self.T // self.TT
        self.TQ = 512                  # attention tq tile
        self.SQT = S // self.TQ        # tq tiles per batch
        self.TPC = self.T // NC        # tokens per core for out-proj
        # out-proj j tile width: widest that fits the recycled qkv SBUF slot
        self.JW = 512 if self.HPC * self.T >= (D // 128) * 512 else 256
        self.NJS = D // self.JW        # out-proj j tiles
        self.NTI = self.TPC // 128     # out-proj token chunks per core
        self.SCALE = float(1.0 / np.sqrt(self.DK))


FULL = Cfg(B=2, S=2048, D=2048, H=16, NC=8)


# --------------------------------------------------------------------------
# host-side prep
# --------------------------------------------------------------------------

def host_prep(cfg, x, w_qkv, w_out, cos, sin):
    B, S, D, H, NC = cfg.B, cfg.S, cfg.D, cfg.H, cfg.NC
    DK, T, HPC, FPC = cfg.DK, cfg.T, cfg.HPC, cfg.FPC

    x = np.asarray(x, dtype=np.float32)
    w_qkv = np.asarray(w_qkv, dtype=np.float32)
    w_out = np.asarray(w_out, dtype=np.float32)
    cos = np.asarray(cos, dtype=np.float32)
    sin = np.asarray(sin, dtype=np.float32)

    xT = np.ascontiguousarray(x.reshape(T, D).T)                  # [D, T]

    # per-head [even; odd] feature permutation for q/k
    perm = np.concatenate([np.arange(0, DK, 2), np.arange(1, DK, 2)])
    w_q, w_k, w_v = w_qkv[0:D], w_qkv[D:2 * D], w_qkv[2 * D:3 * D]

    wqkvT = []
    for c in range(NC):
        rows = slice(FPC * c, FPC * (c + 1))
        wq = w_q[rows].reshape(HPC, DK, D)[:, perm, :].reshape(FPC, D)
        wk = w_k[rows].reshape(HPC, DK, D)[:, perm, :].reshape(FPC, D)
        wv = w_v[rows]
        wqkvT.append(np.ascontiguousarray(np.concatenate([wq, wk, wv], axis=0).T))

    cosT = np.tile(cos.T, (1, B))                                  # [64, T]
    sinT = np.tile(sin.T, (1, B))
    cosF = np.ascontiguousarray(np.concatenate([cosT, cosT], axis=0), dtype=np.float32)
    sinF = np.ascontiguousarray(np.concatenate([-sinT, sinT], axis=0), dtype=np.float32)

    # additive causal masks for S^T diagonal tiles [128, 4*TQ]
    i = np.arange(128)[:, None]
    j = np.arange(cfg.TQ)[None, :]
    masks = np.concatenate(
        [np.where(i <= j - 128 * m, 0.0, NEG).astype(np.float32) for m in range(4)],
        axis=1,
    )
    masks = np.ascontiguousarray(masks)

    woutT = np.ascontiguousarray(w_out.T)                          # [D(f), D(j)]

    ones = np.ones((128, 128), dtype=np.float32)
    shared = dict(xT=xT, cosF=cosF, sinF=sinF, masks=masks, ones=ones, woutT=woutT)
    return shared, wqkvT


# --------------------------------------------------------------------------
# device program
# --------------------------------------------------------------------------

def build_program(cfg):
    nc = bacc.Bacc(
        "TRN2",
        target_bir_lowering=False,
        debug=False,
        num_devices=cfg.NC,
    )

    xT_d = nc.dram_tensor("xT", [cfg.D, cfg.T], F32R, kind="ExternalInput").ap()
    wqkvT_d = nc.dram_tensor("wqkvT", [cfg.D, cfg.W3], F32R, kind="ExternalInput").ap()
    cosF_d = nc.dram_tensor("cosF", [128, cfg.T], F32, kind="ExternalInput").ap()
    sinF_d = nc.dram_tensor("sinF", [128, cfg.T], F32, kind="ExternalInput").ap()
    masks_d = nc.dram_tensor("masks", [128, 4 * cfg.TQ], F32, kind="ExternalInput").ap()
    ones_d = nc.dram_tensor("ones", [128, 128], F32R, kind="ExternalInput").ap()
    woutT_d = nc.dram_tensor("woutT", [cfg.D, cfg.D], F32R, kind="ExternalInput").ap()
    out_d = nc.dram_tensor("out", [cfg.TPC, cfg.D], F32, kind="ExternalOutput").ap()

    with tile.TileContext(nc) as tc:
        _build_body(
            nc, tc, cfg,
            xT_d, wqkvT_d, cosF_d, sinF_d, masks_d, ones_d, woutT_d, out_d,
        )

    nc.compile()
    return nc


def _phase1_qkv_rope(nc, tc, cfg, xT_d, wqkvT_d, cosF_d, sinF_d, q_sb, k_sb, v_sb):
    T, HPC, FPC, W3 = cfg.T, cfg.HPC, cfg.FPC, cfg.W3
    DCH, NTT, TT = cfg.DCH, cfg.NTT, cfg.TT
    VW = 2 * FPC

    with (
        tc.tile_pool(name="wqkv", bufs=1) as wp,
        tc.tile_pool(name="xin", bufs=8) as xp,
        tc.tile_pool(name="csin", bufs=2) as csp,
        tc.tile_pool(name="ropet", bufs=2) as rtp,
        tc.tile_pool(name="swp", bufs=2) as swpp,
        tc.tile_pool(name="pqk", bufs=4, space="PSUM") as pqkp,
        tc.tile_pool(name="pv", bufs=4, space="PSUM") as pvp,
    ):
        w_sb = wp.tile([128, DCH * W3], F32R)
        for dc in range(DCH):
            nc.gpsimd.dma_start(
                w_sb[:, dc * W3:(dc + 1) * W3],
                wqkvT_d[128 * dc:128 * (dc + 1), :],
            )

        for tt in range(NTT):
            cos_t = csp.tile([128, TT], F32, tag="cos")
            nc.sync.dma_start(cos_t[:], cosF_d[:, TT * tt:TT * (tt + 1)])
            sin_t = csp.tile([128, TT], F32, tag="sin")
            nc.sync.dma_start(sin_t[:], sinF_d[:, TT * tt:TT * (tt + 1)])

            xts = []
            for dc in range(DCH):
                x_t = xp.tile([128, TT], F32R, tag="x", name=f"x_{tt}_{dc}")
                nc.sync.dma_start(
                    x_t[:], xT_d[128 * dc:128 * (dc + 1), TT * tt:TT * (tt + 1)]
                )
                xts.append(x_t)

            nqk = 2 * HPC
            pqs = [pqkp.tile([128, TT], F32, tag="qk", name=f"pq{i}") for i in range(nqk)]
            pvs = [pvp.tile([128, FPC], F32, tag="v", name=f"pv_{tt}_{i}")
                   for i in range(4)]

            for dc in range(DCH):
                xr = xts[dc][:]
                first, last = dc == 0, dc == DCH - 1
                for oc in range(nqk):
                    wsl = w_sb[:, dc * W3 + 128 * oc:dc * W3 + 128 * (oc + 1)]
                    nc.tensor.matmul(
                        pqs[oc][:], wsl, xr, start=first, stop=last
                    )
                wv = w_sb[:, dc * W3 + 2 * FPC:dc * W3 + W3]
                for ci in range(4):
                    lhs = xts[dc][:, 128 * ci:128 * (ci + 1)]
                    nc.tensor.matmul(
                        pvs[ci][:], lhs, wv, start=first, stop=last
                    )

            # v: psum -> sbuf token-major per head
            for ci in range(4):
                g = 4 * tt + ci
                for hc in range(HPC):
                    nc.vector.tensor_copy(
                        v_sb[:, hc * T + 128 * g:hc * T + 128 * (g + 1)],
                        pvs[ci][:, 128 * hc:128 * (hc + 1)],
                    )

            # rope: z' = z*cosF + swap(z)*sinF
            for oc in range(nqk):
                dst = q_sb if oc < HPC else k_sb
                hc = oc % HPC
                z = pqs[oc]
                zc = rtp.tile([128, TT], F32, tag="zc")
                nc.vector.tensor_copy(zc[:], z[:])
                zsw = swpp.tile([128, TT], F32, tag="sw")
                nc.sync.dma_start(zsw[0:64, :], zc[64:128, :])
                nc.sync.dma_start(zsw[64:128, :], zc[0:64, :])
                t1 = rtp.tile([128, TT], F32, tag="t1")
                t2 = rtp.tile([128, TT], F32, tag="t2")
                nc.vector.tensor_mul(t1[:], zc[:], cos_t[:])
                nc.vector.tensor_mul(t2[:], zsw[:], sin_t[:])
                nc.vector.tensor_add(
                    dst[:, hc * T + TT * tt:hc * T + TT * (tt + 1)], t1[:], t2[:]
                )


def _phase2_attention(nc, tc, cfg, q_sb, k_sb, v_sb, oT_sb, masks_d, ones_sb):
    T, S, HPC, TQ, SQT = cfg.T, cfg.S, cfg.HPC, cfg.TQ, cfg.SQT

    with (
        tc.tile_pool(name="maskp", bufs=1) as mkp,
        tc.tile_pool(name="pexp", bufs=4) as pep,
        tc.tile_pool(name="linv", bufs=2) as lip,
        tc.tile_pool(name="pss", bufs=4, space="PSUM") as pssp,
        tc.tile_pool(name="pso", bufs=2, space="PSUM") as psop,
        tc.tile_pool(name="psl", bufs=2, space="PSUM") as pslp,
    ):
        masks_sb = mkp.tile([128, 4 * TQ], F32)
        nc.sync.dma_start(masks_sb[:], masks_d[:])
        onesr = ones_sb[:]
        for hc in range(HPC):
            for b in range(cfg.B):
                base = hc * T + S * b
                for jq in range(SQT):
                    o_ps = psop.tile([128, TQ], F32, tag="o", name=f"o_{hc}_{b}_{jq}")
                    l_ps = pslp.tile([128, TQ], F32, tag="l", name=f"l_{hc}_{b}_{jq}")
                    qsl = q_sb[:, base + TQ * jq:base + TQ * (jq + 1)]
                    nkc = 4 * (jq + 1)
                    for ck in range(nkc):
                        s_ps = pssp.tile([128, TQ], F32, tag="s",
                                         name=f"s_{hc}_{b}_{jq}_{ck}")
                        ksl = k_sb[:, base + 128 * ck:base + 128 * (ck + 1)]
                        nc.tensor.matmul(
                            s_ps[:], ksl, qsl, start=True, stop=True
                        )
                        m = ck - 4 * jq
                        if m >= 0:
                            nc.vector.tensor_add(
                                s_ps[:], s_ps[:],
                                masks_sb[:, TQ * m:TQ * (m + 1)],
                            )
                        p_sb = pep.tile([128, TQ], F32R, tag="p",
                                        name=f"p_{hc}_{b}_{jq}_{ck}")
                        nc.scalar.activation(
                            p_sb[:], s_ps[:],
                            mybir.ActivationFunctionType.Exp,
                            scale=cfg.SCALE,
                        )
                        pr = p_sb[:]
                        g = (S // 128) * b + ck
                        vsl = v_sb[:, hc * T + 128 * g:hc * T + 128 * (g + 1)]
                        first, last = ck == 0, ck == nkc - 1
                        nc.tensor.matmul(
                            o_ps[:], vsl, pr, start=first, stop=last
                        )
                        nc.tensor.matmul(
                            l_ps[:], onesr, pr, start=first, stop=last
                        )
                    linv = lip.tile([128, TQ], F32, tag="li", name=f"li_{hc}_{b}_{jq}")
                    nc.vector.reciprocal_approx_fast(linv[:], l_ps[:])
                    nc.vector.tensor_mul(
                        oT_sb[:, base + TQ * jq:base + TQ * (jq + 1)],
                        o_ps[:], linv[:],
                    )


def _phase3_outproj(nc, tc, cfg, qkvp, oT_sb, woutT_d, out_d):
    T, S, HPC, FPC, DCH = cfg.T, cfg.S, cfg.HPC, cfg.FPC, cfg.DCH
    JW = cfg.JW

    with (
        tc.tile_pool(name="dram", bufs=1, space="DRAM") as dramp,
        tc.tile_pool(name="osb", bufs=3) as osbp,
        tc.tile_pool(name="pout", bufs=3, space="PSUM") as poutp,
    ):
        # One A2A per head so the first fires while later heads' attention is
        # still running (collectives run on TOPSP/SDMA, not the 5 engines).
        a2a_outs = []
        for hc in range(HPC):
            ob = dramp.tile([cfg.NC * 128, cfg.TPC], F32R, name=f"obounce{hc}")
            for s in range(cfg.NC):
                t0 = cfg.TPC * s
                b, sb0 = t0 // S, t0 % S
                nc.sync.dma_start(
                    ob[128 * s:128 * (s + 1), :],
                    oT_sb[:, hc * T + S * b + sb0:
                          hc * T + S * b + sb0 + cfg.TPC],
                )
            ao = dramp.tile([cfg.NC * 128, cfg.TPC], F32R, name=f"a2a_out{hc}")
            nc.gpsimd.collective_compute(
                "AllToAll",
                mybir.AluOpType.bypass,
                replica_groups=[list(range(cfg.NC))],
                ins=[ob[:].opt()],
                outs=[ao[:].opt()],
            )
            a2a_outs.append(ao)

        # Recycle the dead q/k/v SBUF slots (tag-shared) for out-proj tiles:
        # rhs (gathered o^T) in v's slot, streamed w_out tiles in q/k's slots.
        rhs_scr = qkvp.tile([128, HPC * T], F32R, tag="v_sb", name="rhs_scr")
        wscr = [
            qkvp.tile([128, HPC * T], F32R, tag="q_sb", name="wscr0"),
            qkvp.tile([128, HPC * T], F32R, tag="k_sb", name="wscr1"),
        ]

        rhs = []
        for fc in range(DCH):
            r_, hc = fc // HPC, fc % HPC
            sl = rhs_scr[:, cfg.TPC * fc:cfg.TPC * (fc + 1)]
            nc.sync.dma_start(sl, a2a_outs[hc][128 * r_:128 * (r_ + 1), :])
            rhs.append(sl)

        for js in range(cfg.NJS):
            ws = wscr[js % 2]
            wts = []
            for fc in range(DCH):
                wt = ws[:, JW * fc:JW * (fc + 1)]
                nc.sync.dma_start(
                    wt,
                    woutT_d[128 * fc:128 * (fc + 1), JW * js:JW * (js + 1)],
                )
                wts.append(wt)
            for ti in range(cfg.NTI):
                ps = poutp.tile([128, JW], F32, tag="po", name=f"po_{js}_{ti}")
                for fc in range(DCH):
                    nc.tensor.matmul(
                        ps[:],
                        rhs[fc][:, 128 * ti:128 * (ti + 1)],
                        wts[fc],
                        start=(fc == 0), stop=(fc == DCH - 1),
                    )
                osb = osbp.tile([128, JW], F32, tag="ob", name=f"ob_{js}_{ti}")
                nc.vector.tensor_copy(osb[:], ps[:])
                nc.sync.dma_start(
                    out_d[128 * ti:128 * (ti + 1), JW * js:JW * (js + 1)],
                    osb[:],
                )


def _build_body(nc, tc, cfg, xT_d, wqkvT_d, cosF_d, sinF_d, masks_d, ones_d, woutT_d, out_d):
    T, HPC, TQ = cfg.T, cfg.HPC, cfg.TQ

    with tc.tile_pool(name="const", bufs=1) as constp:
        ones_sb = constp.tile([128, 128], F32R)
        nc.sync.dma_start(ones_sb[:], ones_d[:])

        with tc.tile_pool(name="qkvp", bufs=1) as qkvp:
            q_sb = qkvp.tile([128, HPC * T], F32R)
            k_sb = qkvp.tile([128, HPC * T], F32R)
            v_sb = qkvp.tile([128, HPC * T], F32R)

            _phase1_qkv_rope(
                nc, tc, cfg, xT_d, wqkvT_d, cosF_d, sinF_d, q_sb, k_sb, v_sb
            )

            with tc.tile_pool(name="oT", bufs=1) as otp:
                oT_sb = otp.tile([128, HPC * T], F32R)

                _phase2_attention(
                    nc, tc, cfg, q_sb, k_sb, v_sb, oT_sb, masks_d, ones_sb
                )
                _phase3_outproj(nc, tc, cfg, qkvp, oT_sb, woutT_d, out_d)


# --------------------------------------------------------------------------
# host entry point
# --------------------------------------------------------------------------

_CACHE = {}


def _compiled(cfg):
    key = (cfg.B, cfg.S, cfg.D, cfg.H, cfg.NC)
    if key not in _CACHE:
        _CACHE[key] = build_program(cfg)
    return _CACHE[key]


def make_in_maps(cfg, inputs):
    shared, wqkvT = host_prep(
        cfg, inputs["x"], inputs["w_qkv"], inputs["w_out"],
        inputs["cos"], inputs["sin"],
    )
    return [{**shared, "wqkvT": wqkvT[c]} for c in range(cfg.NC)]


def assemble(cfg, results):
    out = np.concatenate([results[c]["out"] for c in range(cfg.NC)], axis=0)
    return out.reshape(cfg.B, cfg.S, cfg.D).astype(np.float32)


def kernel(x, w_qkv, w_out, cos, sin):
    cfg = FULL
    nc = _compiled(cfg)
    in_maps = make_in_maps(cfg, dict(x=x, w_qkv=w_qkv, w_out=w_out, cos=cos, sin=sin))
    res = bass_utils.run_bass_kernel_spmd(nc, in_maps, core_ids=list(range(cfg.NC)))
    return assemble(cfg, res.results)



# revision 2
# speedup vs baseline: 1.1337x; 1.1337x over previous
"""Causal MHA (B=2, S=2048, D=2048, H=16) on 8 trn2 NeuronCores.

Sharding: tensor-parallel over heads. Each core computes QKV + RoPE + causal
SDPA for H/8 heads end-to-end, then an AllToAll redistributes attention
outputs from head-sharded to token-sharded layout, and each core computes the
full out-projection for its 1/8 token slice (avoids the 32MB AllReduce the
naive row-parallel scheme would need; A2A moves only 2MB/core in bf16).

Device layouts (everything feature-major, partition dim = 128):
  xT      [D, T]        x transposed (T = B*S tokens, b-major), bf16
  q/k/v   [128, HPC*T]  bf16 per-core; q/k feature rows are per-head permuted
                        to [even(64); odd(64)] so RoPE's pair rotation becomes
                        a 64-partition block swap (done via 2 small DMAs)
  scores  S^T tiles [tk=128, tq=512]: softmax denominator comes from an
                        all-ones lhsT matmul accumulated alongside P@V
  out-proj: w_out resident in SBUF (bf16, preloaded during phases 1-2);
            two-round accumulation (even-head chunks after the first A2A,
            odd-head chunks after the second) so the PE keeps working while
            the second A2A is in flight. out[t, j] token-major so the host
            just concatenates core blocks.

All matmul inputs are bf16 (1 col/cycle on the PE, same rate as float32r,
but half the LDWEIGHTS time, SBUF footprint, DMA and collective bytes);
accumulation stays fp32 in PSUM.
"""

import ml_dtypes
import numpy as np

import concourse.bass as bass
import concourse.bacc as bacc
import concourse.mybir as mybir
import concourse.tile as tile
from concourse import bass_utils

F32 = mybir.dt.float32
BF16 = mybir.dt.bfloat16
NEG = -1e9
BF16NP = ml_dtypes.bfloat16


class Cfg:
    def __init__(self, B, S, D, H, NC=8):
        self.B, self.S, self.D, self.H, self.NC = B, S, D, H, NC
        self.DK = D // H
        assert self.DK == 128, "kernel assumes head dim 128"
        self.T = B * S                 # tokens, b-major
        self.HPC = H // NC             # heads per core
        self.FPC = self.HPC * self.DK  # features per core (q or k or v)
        self.W3 = 3 * self.FPC
        self.DCH = D // 128            # contraction chunks
        self.TT = 512                  # qkv token tile
        self.NTT = self.T // self.TT
        self.TQ = 512                  # attention tq tile
        self.SQT = S // self.TQ        # tq tiles per batch
        self.TPC = self.T // NC        # tokens per core for out-proj
        self.JW = 512                  # out-proj j tile width
        self.NJS = D // self.JW        # out-proj j tiles
        self.NTI = self.TPC // 128     # out-proj token chunks per core
        self.SCALE = float(1.0 / np.sqrt(self.DK))


FULL = Cfg(B=2, S=2048, D=2048, H=16, NC=8)


# --------------------------------------------------------------------------
# host-side prep
# --------------------------------------------------------------------------

def host_prep(cfg, x, w_qkv, w_out, cos, sin):
    B, S, D, H, NC = cfg.B, cfg.S, cfg.D, cfg.H, cfg.NC
    DK, T, HPC, FPC = cfg.DK, cfg.T, cfg.HPC, cfg.FPC

    x = np.asarray(x, dtype=np.float32)
    w_qkv = np.asarray(w_qkv, dtype=np.float32)
    w_out = np.asarray(w_out, dtype=np.float32)
    cos = np.asarray(cos, dtype=np.float32)
    sin = np.asarray(sin, dtype=np.float32)

    xT = np.ascontiguousarray(x.reshape(T, D).T).astype(BF16NP)    # [D, T]

    # per-head [even; odd] feature permutation for q/k
    perm = np.concatenate([np.arange(0, DK, 2), np.arange(1, DK, 2)])
    w_q, w_k, w_v = w_qkv[0:D], w_qkv[D:2 * D], w_qkv[2 * D:3 * D]

    wqkvT = []
    for c in range(NC):
        rows = slice(FPC * c, FPC * (c + 1))
        wq = w_q[rows].reshape(HPC, DK, D)[:, perm, :].reshape(FPC, D)
        wk = w_k[rows].reshape(HPC, DK, D)[:, perm, :].reshape(FPC, D)
        wv = w_v[rows]
        wqkvT.append(np.ascontiguousarray(
            np.concatenate([wq, wk, wv], axis=0).T).astype(BF16NP))

    cosT = np.tile(cos.T, (1, B))                                  # [64, T]
    sinT = np.tile(sin.T, (1, B))
    cosF = np.ascontiguousarray(np.concatenate([cosT, cosT], axis=0), dtype=np.float32)
    sinF = np.ascontiguousarray(np.concatenate([-sinT, sinT], axis=0), dtype=np.float32)

    # additive causal masks for S^T diagonal tiles [128, 4*TQ]
    i = np.arange(128)[:, None]
    j = np.arange(cfg.TQ)[None, :]
    masks = np.concatenate(
        [np.where(i <= j - 128 * m, 0.0, NEG).astype(np.float32) for m in range(4)],
        axis=1,
    )
    masks = np.ascontiguousarray(masks)

    woutT = np.ascontiguousarray(w_out.T).astype(BF16NP)           # [D(f), D(j)]

    ones = np.ones((128, 128), dtype=BF16NP)
    shared = dict(xT=xT, cosF=cosF, sinF=sinF, masks=masks, ones=ones, woutT=woutT)
    return shared, wqkvT


# --------------------------------------------------------------------------
# device program
# --------------------------------------------------------------------------

def build_program(cfg):
    nc = bacc.Bacc(
        "TRN2",
        target_bir_lowering=False,
        debug=False,
        num_devices=cfg.NC,
    )

    xT_d = nc.dram_tensor("xT", [cfg.D, cfg.T], BF16, kind="ExternalInput").ap()
    wqkvT_d = nc.dram_tensor("wqkvT", [cfg.D, cfg.W3], BF16, kind="ExternalInput").ap()
    cosF_d = nc.dram_tensor("cosF", [128, cfg.T], F32, kind="ExternalInput").ap()
    sinF_d = nc.dram_tensor("sinF", [128, cfg.T], F32, kind="ExternalInput").ap()
    masks_d = nc.dram_tensor("masks", [128, 4 * cfg.TQ], F32, kind="ExternalInput").ap()
    ones_d = nc.dram_tensor("ones", [128, 128], BF16, kind="ExternalInput").ap()
    woutT_d = nc.dram_tensor("woutT", [cfg.D, cfg.D], BF16, kind="ExternalInput").ap()
    out_d = nc.dram_tensor("out", [cfg.TPC, cfg.D], F32, kind="ExternalOutput").ap()

    with tile.TileContext(nc) as tc:
        _build_body(
            nc, tc, cfg,
            xT_d, wqkvT_d, cosF_d, sinF_d, masks_d, ones_d, woutT_d, out_d,
        )

    nc.compile()
    return nc


def _phase1_qkv_rope(nc, tc, cfg, xT_d, wqkvT_d, cosF_d, sinF_d, q_sb, k_sb, v_sb):
    T, HPC, FPC, W3 = cfg.T, cfg.HPC, cfg.FPC, cfg.W3
    DCH, NTT, TT = cfg.DCH, cfg.NTT, cfg.TT

    with (
        tc.tile_pool(name="wqkv", bufs=1) as wp,
        tc.tile_pool(name="xin", bufs=8) as xp,
        tc.tile_pool(name="csin", bufs=2) as csp,
        tc.tile_pool(name="ropet", bufs=2) as rtp,
        tc.tile_pool(name="swp", bufs=2) as swpp,
        tc.tile_pool(name="pqk", bufs=4, space="PSUM") as pqkp,
        tc.tile_pool(name="pv", bufs=4, space="PSUM") as pvp,
    ):
        w_sb = wp.tile([128, DCH * W3], BF16)
        for dc in range(DCH):
            nc.gpsimd.dma_start(
                w_sb[:, dc * W3:(dc + 1) * W3],
                wqkvT_d[128 * dc:128 * (dc + 1), :],
            )

        for tt in range(NTT):
            cos_t = csp.tile([128, TT], F32, tag="cos")
            nc.sync.dma_start(cos_t[:], cosF_d[:, TT * tt:TT * (tt + 1)])
            sin_t = csp.tile([128, TT], F32, tag="sin")
            nc.sync.dma_start(sin_t[:], sinF_d[:, TT * tt:TT * (tt + 1)])

            xts = []
            for dc in range(DCH):
                x_t = xp.tile([128, TT], BF16, tag="x", name=f"x_{tt}_{dc}")
                nc.sync.dma_start(
                    x_t[:], xT_d[128 * dc:128 * (dc + 1), TT * tt:TT * (tt + 1)]
                )
                xts.append(x_t)

            nqk = 2 * HPC
            pqs = [pqkp.tile([128, TT], F32, tag="qk", name=f"pq{i}") for i in range(nqk)]
            pvs = [pvp.tile([128, FPC], F32, tag="v", name=f"pv_{tt}_{i}")
                   for i in range(4)]

            for dc in range(DCH):
                xr = xts[dc][:]
                first, last = dc == 0, dc == DCH - 1
                for oc in range(nqk):
                    wsl = w_sb[:, dc * W3 + 128 * oc:dc * W3 + 128 * (oc + 1)]
                    nc.tensor.matmul(
                        pqs[oc][:], wsl, xr, start=first, stop=last
                    )
                wv = w_sb[:, dc * W3 + 2 * FPC:dc * W3 + W3]
                for ci in range(4):
                    lhs = xts[dc][:, 128 * ci:128 * (ci + 1)]
                    nc.tensor.matmul(
                        pvs[ci][:], lhs, wv, start=first, stop=last
                    )

            # v: psum -> sbuf token-major per head (cast to bf16)
            for ci in range(4):
                g = 4 * tt + ci
                for hc in range(HPC):
                    nc.vector.tensor_copy(
                        v_sb[:, hc * T + 128 * g:hc * T + 128 * (g + 1)],
                        pvs[ci][:, 128 * hc:128 * (hc + 1)],
                    )

            # rope: z' = z*cosF + swap(z)*sinF
            for oc in range(nqk):
                dst = q_sb if oc < HPC else k_sb
                hc = oc % HPC
                z = pqs[oc]
                zc = rtp.tile([128, TT], F32, tag="zc")
                nc.vector.tensor_copy(zc[:], z[:])
                zsw = swpp.tile([128, TT], F32, tag="sw")
                nc.sync.dma_start(zsw[0:64, :], zc[64:128, :])
                nc.sync.dma_start(zsw[64:128, :], zc[0:64, :])
                t1 = rtp.tile([128, TT], F32, tag="t1")
                t2 = rtp.tile([128, TT], F32, tag="t2")
                nc.vector.tensor_mul(t1[:], zc[:], cos_t[:])
                nc.vector.tensor_mul(t2[:], zsw[:], sin_t[:])
                nc.vector.tensor_add(
                    dst[:, hc * T + TT * tt:hc * T + TT * (tt + 1)], t1[:], t2[:]
                )


def _phase2_attention(nc, tc, cfg, q_sb, k_sb, v_sb, oT_sb, masks_d, ones_sb):
    T, S, HPC, TQ, SQT = cfg.T, cfg.S, cfg.HPC, cfg.TQ, cfg.SQT

    with (
        tc.tile_pool(name="maskp", bufs=1) as mkp,
        tc.tile_pool(name="pexp", bufs=4) as pep,
        tc.tile_pool(name="linv", bufs=2) as lip,
        tc.tile_pool(name="pss", bufs=4, space="PSUM") as pssp,
        tc.tile_pool(name="pso", bufs=2, space="PSUM") as psop,
        tc.tile_pool(name="psl", bufs=2, space="PSUM") as pslp,
    ):
        masks_sb = mkp.tile([128, 4 * TQ], F32)
        nc.sync.dma_start(masks_sb[:], masks_d[:])
        onesr = ones_sb[:]
        for hc in range(HPC):
            for b in range(cfg.B):
                base = hc * T + S * b
                for jq in range(SQT):
                    o_ps = psop.tile([128, TQ], F32, tag="o", name=f"o_{hc}_{b}_{jq}")
                    l_ps = pslp.tile([128, TQ], F32, tag="l", name=f"l_{hc}_{b}_{jq}")
                    qsl = q_sb[:, base + TQ * jq:base + TQ * (jq + 1)]
                    nkc = 4 * (jq + 1)
                    for ck in range(nkc):
                        s_ps = pssp.tile([128, TQ], F32, tag="s",
                                         name=f"s_{hc}_{b}_{jq}_{ck}")
                        ksl = k_sb[:, base + 128 * ck:base + 128 * (ck + 1)]
                        nc.tensor.matmul(
                            s_ps[:], ksl, qsl, start=True, stop=True
                        )
                        m = ck - 4 * jq
                        if m >= 0:
                            nc.vector.tensor_add(
                                s_ps[:], s_ps[:],
                                masks_sb[:, TQ * m:TQ * (m + 1)],
                            )
                        p_sb = pep.tile([128, TQ], BF16, tag="p",
                                        name=f"p_{hc}_{b}_{jq}_{ck}")
                        nc.scalar.activation(
                            p_sb[:], s_ps[:],
                            mybir.ActivationFunctionType.Exp,
                            scale=cfg.SCALE,
                        )
                        pr = p_sb[:]
                        g = (S // 128) * b + ck
                        vsl = v_sb[:, hc * T + 128 * g:hc * T + 128 * (g + 1)]
                        first, last = ck == 0, ck == nkc - 1
                        nc.tensor.matmul(
                            o_ps[:], vsl, pr, start=first, stop=last
                        )
                        nc.tensor.matmul(
                            l_ps[:], onesr, pr, start=first, stop=last
                        )
                    linv = lip.tile([128, TQ], F32, tag="li", name=f"li_{hc}_{b}_{jq}")
                    nc.vector.reciprocal_approx_fast(linv[:], l_ps[:])
                    nc.vector.tensor_mul(
                        oT_sb[:, base + TQ * jq:base + TQ * (jq + 1)],
                        o_ps[:], linv[:],
                    )


def _phase3_outproj(nc, tc, cfg, qkvp, oT_sb, wout_sb, out_d):
    T, S, HPC, DCH, TPC = cfg.T, cfg.S, cfg.HPC, cfg.DCH, cfg.TPC
    D, JW = cfg.D, cfg.JW

    with (
        tc.tile_pool(name="dram", bufs=1, space="DRAM") as dramp,
        tc.tile_pool(name="osb", bufs=3) as osbp,
        tc.tile_pool(name="pout", bufs=3, space="PSUM") as poutp,
    ):
        # One A2A per head so the first fires while the second head's
        # attention is still running (collectives run on CC cores/links,
        # not the 5 engines). Payload is bf16: 1MB per head per core.
        a2a_outs = []
        for hc in range(HPC):
            ob = dramp.tile([cfg.NC * 128, TPC], BF16, name=f"obounce{hc}")
            for s in range(cfg.NC):
                t0 = TPC * s
                b, sb0 = t0 // S, t0 % S
                nc.sync.dma_start(
                    ob[128 * s:128 * (s + 1), :],
                    oT_sb[:, hc * T + S * b + sb0:
                          hc * T + S * b + sb0 + TPC],
                )
            ao = dramp.tile([cfg.NC * 128, TPC], BF16, name=f"a2a_out{hc}")
            nc.gpsimd.collective_compute(
                "AllToAll",
                mybir.AluOpType.bypass,
                replica_groups=[list(range(cfg.NC))],
                ins=[ob[:].opt()],
                outs=[ao[:].opt()],
            )
            a2a_outs.append(ao)

        # Recycle the dead q/k/v SBUF slots (tag-shared): gathered o^T (bf16)
        # in v's slot, fp32 round-A partials in q's and k's slots.
        rhs_scr = qkvp.tile([128, DCH * TPC], BF16, tag="v_sb", name="rhs_scr")
        part_q = qkvp.tile([128, 8 * JW], F32, tag="q_sb", name="part_q")
        part_k = qkvp.tile([128, 8 * JW], F32, tag="k_sb", name="part_k")

        def part(js, ti):
            p = 4 * js + ti
            base = part_q if p < 8 else part_k
            return base[:, JW * (p % 8):JW * (p % 8 + 1)]

        rhs = [rhs_scr[:, TPC * fc:TPC * (fc + 1)] for fc in range(DCH)]
        for rnd in range(2):  # rnd 0: even heads (first A2A); 1: odd heads
            fcs = [fc for fc in range(DCH) if fc % HPC == rnd]
            for fc in fcs:
                nc.sync.dma_start(
                    rhs[fc], a2a_outs[rnd][128 * (fc // HPC):128 * (fc // HPC + 1), :]
                )
            for js in range(cfg.NJS):
                for ti in range(cfg.NTI):
                    ps = poutp.tile([128, JW], F32, tag="po",
                                    name=f"po_{rnd}_{js}_{ti}")
                    for i, fc in enumerate(fcs):
                        nc.tensor.matmul(
                            ps[:],
                            rhs[fc][:, 128 * ti:128 * (ti + 1)],
                            wout_sb[:, D * fc + JW * js:D * fc + JW * (js + 1)],
                            start=(i == 0), stop=(i == len(fcs) - 1),
                        )
                    if rnd == 0:
                        nc.vector.tensor_copy(part(js, ti), ps[:])
                    else:
                        osb = osbp.tile([128, JW], F32, tag="ob",
                                        name=f"ob_{js}_{ti}")
                        nc.vector.tensor_add(osb[:], ps[:], part(js, ti))
                        nc.sync.dma_start(
                            out_d[128 * ti:128 * (ti + 1), JW * js:JW * (js + 1)],
                            osb[:],
                        )


def _build_body(nc, tc, cfg, xT_d, wqkvT_d, cosF_d, sinF_d, masks_d, ones_d, woutT_d, out_d):
    T, HPC, D, DCH = cfg.T, cfg.HPC, cfg.D, cfg.DCH

    with tc.tile_pool(name="const", bufs=1) as constp:
        ones_sb = constp.tile([128, 128], BF16)
        nc.sync.dma_start(ones_sb[:], ones_d[:])

        with (
            tc.tile_pool(name="woutp", bufs=1) as wop,
            tc.tile_pool(name="qkvp", bufs=1) as qkvp,
        ):
            # preload w_out (bf16, 64KB/partition) on the scalar DMA ring so
            # it's resident long before the out-projection needs it and never
            # contends with the x loads (sync ring) or w_qkv (gpsimd ring).
            wout_sb = wop.tile([128, DCH * D], BF16)
            for fc in range(DCH):
                nc.scalar.dma_start(
                    wout_sb[:, D * fc:D * (fc + 1)],
                    woutT_d[128 * fc:128 * (fc + 1), :],
                )

            q_sb = qkvp.tile([128, HPC * T], BF16)
            k_sb = qkvp.tile([128, HPC * T], BF16)
            v_sb = qkvp.tile([128, HPC * T], BF16)

            _phase1_qkv_rope(
                nc, tc, cfg, xT_d, wqkvT_d, cosF_d, sinF_d, q_sb, k_sb, v_sb
            )

            with tc.tile_pool(name="oT", bufs=1) as otp:
                oT_sb = otp.tile([128, HPC * T], BF16)

                _phase2_attention(
                    nc, tc, cfg, q_sb, k_sb, v_sb, oT_sb, masks_d, ones_sb
                )
                _phase3_outproj(nc, tc, cfg, qkvp, oT_sb, wout_sb, out_d)


# --------------------------------------------------------------------------
# host entry point
# --------------------------------------------------------------------------

_CACHE = {}


def _compiled(cfg):
    key = (cfg.B, cfg.S, cfg.D, cfg.H, cfg.NC)
    if key not in _CACHE:
        _CACHE[key] = build_program(cfg)
    return _CACHE[key]


def make_in_maps(cfg, inputs):
    shared, wqkvT = host_prep(
        cfg, inputs["x"], inputs["w_qkv"], inputs["w_out"],
        inputs["cos"], inputs["sin"],
    )
    return [{**shared, "wqkvT": wqkvT[c]} for c in range(cfg.NC)]


def assemble(cfg, results):
    out = np.concatenate([results[c]["out"] for c in range(cfg.NC)], axis=0)
    return out.reshape(cfg.B, cfg.S, cfg.D).astype(np.float32)


def kernel(x, w_qkv, w_out, cos, sin):
    cfg = FULL
    nc = _compiled(cfg)
    in_maps = make_in_maps(cfg, dict(x=x, w_qkv=w_qkv, w_out=w_out, cos=cos, sin=sin))
    res = bass_utils.run_bass_kernel_spmd(nc, in_maps, core_ids=list(range(cfg.NC)))
    return assemble(cfg, res.results)


# revision 8
# speedup vs baseline: 1.1894x; 1.0492x over previous
"""Causal MHA (B=2, S=2048, D=2048, H=16) on 8 trn2 NeuronCores.

Sharding: tensor-parallel over heads. Each core computes QKV + RoPE + causal
SDPA for H/8 heads end-to-end, then an AllToAll redistributes attention
outputs from head-sharded to token-sharded layout, and each core computes the
full out-projection for its 1/8 token slice (avoids the 32MB AllReduce the
naive row-parallel scheme would need; A2A moves only 2MB/core in bf16).

Device layouts (everything feature-major, partition dim = 128):
  xT      [D, T]        x transposed (T = B*S tokens, b-major), bf16
  q/k/v   [128, HPC*T]  bf16 per-core; q/k feature rows are per-head permuted
                        to [even(64); odd(64)] so RoPE's pair rotation becomes
                        a 64-partition block swap (done via 2 small DMAs)
  scores  S^T tiles [tk=128, tq=512]: softmax denominator comes from an
                        all-ones lhsT matmul accumulated alongside P@V
  out-proj: w_out resident in SBUF (bf16, preloaded during phases 1-2);
            two-round accumulation (even-head chunks after the first A2A,
            odd-head chunks after the second) so the PE keeps working while
            the second A2A is in flight. out[t, j] token-major so the host
            just concatenates core blocks.

All matmul inputs are bf16 (1 col/cycle on the PE, same rate as float32r,
but half the LDWEIGHTS time, SBUF footprint, DMA and collective bytes);
accumulation stays fp32 in PSUM.
"""

import ml_dtypes
import numpy as np

import concourse.bass as bass
import concourse.bacc as bacc
import concourse.mybir as mybir
import concourse.tile as tile
from concourse import bass_utils

F32 = mybir.dt.float32
BF16 = mybir.dt.bfloat16
NEG = -1e9
BF16NP = ml_dtypes.bfloat16


class Cfg:
    def __init__(self, B, S, D, H, NC=8):
        self.B, self.S, self.D, self.H, self.NC = B, S, D, H, NC
        self.DK = D // H
        assert self.DK == 128, "kernel assumes head dim 128"
        self.T = B * S                 # tokens, b-major
        self.HPC = H // NC             # heads per core
        self.FPC = self.HPC * self.DK  # features per core (q or k or v)
        self.W3 = 3 * self.FPC
        self.DCH = D // 128            # contraction chunks
        self.TT = 512                  # qkv token tile
        self.NTT = self.T // self.TT
        self.TQ = 512                  # attention tq tile
        self.SQT = S // self.TQ        # tq tiles per batch
        self.TPC = self.T // NC        # tokens per core for out-proj
        self.JW = 512                  # out-proj j tile width
        self.NJS = D // self.JW        # out-proj j tiles
        self.NTI = self.TPC // 128     # out-proj token chunks per core
        self.SCALE = float(1.0 / np.sqrt(self.DK))


FULL = Cfg(B=2, S=2048, D=2048, H=16, NC=8)


# --------------------------------------------------------------------------
# host-side prep
# --------------------------------------------------------------------------

def host_prep(cfg, x, w_qkv, w_out, cos, sin):
    B, S, D, H, NC = cfg.B, cfg.S, cfg.D, cfg.H, cfg.NC
    DK, T, HPC, FPC = cfg.DK, cfg.T, cfg.HPC, cfg.FPC

    x = np.asarray(x, dtype=np.float32)
    w_qkv = np.asarray(w_qkv, dtype=np.float32)
    w_out = np.asarray(w_out, dtype=np.float32)
    cos = np.asarray(cos, dtype=np.float32)
    sin = np.asarray(sin, dtype=np.float32)

    xT = np.ascontiguousarray(x.reshape(T, D).T).astype(BF16NP)    # [D, T]

    # per-head [even; odd] feature permutation for q/k
    perm = np.concatenate([np.arange(0, DK, 2), np.arange(1, DK, 2)])
    w_q, w_k, w_v = w_qkv[0:D], w_qkv[D:2 * D], w_qkv[2 * D:3 * D]

    wqkvT = []
    for c in range(NC):
        rows = slice(FPC * c, FPC * (c + 1))
        wq = w_q[rows].reshape(HPC, DK, D)[:, perm, :].reshape(FPC, D)
        wk = w_k[rows].reshape(HPC, DK, D)[:, perm, :].reshape(FPC, D)
        wv = w_v[rows]
        wqkvT.append(np.ascontiguousarray(
            np.concatenate([wq, wk, wv], axis=0).T).astype(BF16NP))

    cosT = np.tile(cos.T, (1, B))                                  # [64, T]
    sinT = np.tile(sin.T, (1, B))
    cosF = np.ascontiguousarray(np.concatenate([cosT, cosT], axis=0), dtype=np.float32)
    sinF = np.ascontiguousarray(np.concatenate([-sinT, sinT], axis=0), dtype=np.float32)

    # additive causal masks for S^T diagonal tiles [128, 4*TQ]
    i = np.arange(128)[:, None]
    j = np.arange(cfg.TQ)[None, :]
    masks = np.concatenate(
        [np.where(i <= j - 128 * m, 0.0, NEG).astype(np.float32) for m in range(4)],
        axis=1,
    )
    masks = np.ascontiguousarray(masks)

    woutT = np.ascontiguousarray(w_out.T).astype(BF16NP)           # [D(f), D(j)]

    ones = np.ones((128, 128), dtype=BF16NP)
    shared = dict(xT=xT, cosF=cosF, sinF=sinF, masks=masks, ones=ones, woutT=woutT)
    return shared, wqkvT


# --------------------------------------------------------------------------
# device program
# --------------------------------------------------------------------------

def build_program(cfg):
    nc = bacc.Bacc(
        "TRN2",
        target_bir_lowering=False,
        debug=False,
        num_devices=cfg.NC,
    )

    xT_d = nc.dram_tensor("xT", [cfg.D, cfg.T], BF16, kind="ExternalInput").ap()
    wqkvT_d = nc.dram_tensor("wqkvT", [cfg.D, cfg.W3], BF16, kind="ExternalInput").ap()
    cosF_d = nc.dram_tensor("cosF", [128, cfg.T], F32, kind="ExternalInput").ap()
    sinF_d = nc.dram_tensor("sinF", [128, cfg.T], F32, kind="ExternalInput").ap()
    masks_d = nc.dram_tensor("masks", [128, 4 * cfg.TQ], F32, kind="ExternalInput").ap()
    ones_d = nc.dram_tensor("ones", [128, 128], BF16, kind="ExternalInput").ap()
    woutT_d = nc.dram_tensor("woutT", [cfg.D, cfg.D], BF16, kind="ExternalInput").ap()
    out_d = nc.dram_tensor("out", [cfg.TPC, cfg.D], BF16, kind="ExternalOutput").ap()

    with tile.TileContext(nc) as tc:
        _build_body(
            nc, tc, cfg,
            xT_d, wqkvT_d, cosF_d, sinF_d, masks_d, ones_d, woutT_d, out_d,
        )

    nc.compile()
    return nc


def _phase1_qkv_rope(nc, tc, cfg, xT_d, wqkvT_d, cosF_d, sinF_d,
                     woutT_d, wout_sb, q_sb, k_sb, v_sb):
    T, HPC, FPC, W3 = cfg.T, cfg.HPC, cfg.FPC, cfg.W3
    DCH, NTT, TT, D = cfg.DCH, cfg.NTT, cfg.TT, cfg.D

    with (
        tc.tile_pool(name="wqkv", bufs=1) as wp,
        tc.tile_pool(name="xin", bufs=8) as xp,
        tc.tile_pool(name="csin", bufs=2) as csp,
        tc.tile_pool(name="zcp", bufs=2) as zcp,
        tc.tile_pool(name="ropet", bufs=2) as rtp,
        tc.tile_pool(name="swp", bufs=2) as swpp,
        tc.tile_pool(name="pqk", bufs=4, space="PSUM") as pqkp,
        tc.tile_pool(name="pv", bufs=4, space="PSUM") as pvp,
    ):
        w_sb = wp.tile([128, DCH * W3], BF16)
        for dc in range(DCH):
            nc.gpsimd.dma_start(
                w_sb[:, dc * W3:(dc + 1) * W3],
                wqkvT_d[128 * dc:128 * (dc + 1), :],
            )

        for tt in range(NTT):
            xts = []
            for dc in range(DCH):
                x_t = xp.tile([128, TT], BF16, tag="x", name=f"x_{tt}_{dc}")
                nc.sync.dma_start(
                    x_t[:], xT_d[128 * dc:128 * (dc + 1), TT * tt:TT * (tt + 1)]
                )
                xts.append(x_t)

            cos_t = csp.tile([128, TT], F32, tag="cos")
            nc.sync.dma_start(cos_t[:], cosF_d[:, TT * tt:TT * (tt + 1)])
            sin_t = csp.tile([128, TT], F32, tag="sin")
            nc.sync.dma_start(sin_t[:], sinF_d[:, TT * tt:TT * (tt + 1)])

            # stream two w_out chunks per token tile on the same (sync) DMA
            # ring, strictly behind this tile's x loads, so the preload can
            # never delay phase 1's critical path.
            for fc in (2 * tt, 2 * tt + 1):
                nc.sync.dma_start(
                    wout_sb[:, D * fc:D * (fc + 1)],
                    woutT_d[128 * fc:128 * (fc + 1), :],
                )

            nqk = 2 * HPC
            pqs = [pqkp.tile([128, TT], F32, tag="qk", name=f"pq{i}") for i in range(nqk)]
            pvs = [pvp.tile([128, FPC], F32, tag="v", name=f"pv_{tt}_{i}")
                   for i in range(4)]

            for dc in range(DCH):
                xr = xts[dc][:]
                first, last = dc == 0, dc == DCH - 1
                for oc in range(nqk):
                    wsl = w_sb[:, dc * W3 + 128 * oc:dc * W3 + 128 * (oc + 1)]
                    nc.tensor.matmul(
                        pqs[oc][:], wsl, xr, start=first, stop=last
                    )
                wv = w_sb[:, dc * W3 + 2 * FPC:dc * W3 + W3]
                for ci in range(4):
                    lhs = xts[dc][:, 128 * ci:128 * (ci + 1)]
                    nc.tensor.matmul(
                        pvs[ci][:], lhs, wv, start=first, stop=last
                    )

            # v: psum -> sbuf token-major per head (cast to bf16)
            for ci in range(4):
                g = 4 * tt + ci
                for hc in range(HPC):
                    nc.vector.tensor_copy(
                        v_sb[:, hc * T + 128 * g:hc * T + 128 * (g + 1)],
                        pvs[ci][:, 128 * hc:128 * (hc + 1)],
                    )

            # rope: z' = z*cosF + swap(z)*sinF. All psum->sbuf copies are
            # emitted first so the qk psum banks free as early as possible
            # (they gate the next token tile's matmuls and, at the end of
            # phase 1, the first attention matmuls via bank reuse).
            zcs = []
            for oc in range(nqk):
                zc = zcp.tile([128, TT], F32, tag=f"zc{oc}")
                nc.vector.tensor_copy(zc[:], pqs[oc][:])
                zcs.append(zc)
            for oc in range(nqk):
                dst = q_sb if oc < HPC else k_sb
                hc = oc % HPC
                zc = zcs[oc]
                zsw = swpp.tile([128, TT], F32, tag="sw")
                nc.sync.dma_start(zsw[0:64, :], zc[64:128, :])
                nc.sync.dma_start(zsw[64:128, :], zc[0:64, :])
                t1 = rtp.tile([128, TT], F32, tag="t1")
                t2 = rtp.tile([128, TT], F32, tag="t2")
                nc.vector.tensor_mul(t1[:], zc[:], cos_t[:])
                nc.vector.tensor_mul(t2[:], zsw[:], sin_t[:])
                nc.vector.tensor_add(
                    dst[:, hc * T + TT * tt:hc * T + TT * (tt + 1)], t1[:], t2[:]
                )


def _phase2_attention(nc, tc, cfg, q_sb, k_sb, v_sb, oT_sb, masks_d, ones_sb):
    T, S, HPC, TQ, SQT = cfg.T, cfg.S, cfg.HPC, cfg.TQ, cfg.SQT

    with (
        tc.tile_pool(name="maskp", bufs=1) as mkp,
        tc.tile_pool(name="pexp", bufs=4) as pep,
        tc.tile_pool(name="linv", bufs=2) as lip,
        tc.tile_pool(name="pss", bufs=4, space="PSUM") as pssp,
        tc.tile_pool(name="pso", bufs=2, space="PSUM") as psop,
        tc.tile_pool(name="psl", bufs=2, space="PSUM") as pslp,
    ):
        masks_sb = mkp.tile([128, 4 * TQ], F32)
        nc.sync.dma_start(masks_sb[:], masks_d[:])
        onesr = ones_sb[:]
        for hc in range(HPC):
            for b in range(cfg.B):
                base = hc * T + S * b
                for jq in range(SQT):
                    o_ps = psop.tile([128, TQ], F32, tag="o", name=f"o_{hc}_{b}_{jq}")
                    l_ps = pslp.tile([128, TQ], F32, tag="l", name=f"l_{hc}_{b}_{jq}")
                    qsl = q_sb[:, base + TQ * jq:base + TQ * (jq + 1)]
                    nkc = 4 * (jq + 1)
                    for ck in range(nkc):
                        s_ps = pssp.tile([128, TQ], F32, tag="s",
                                         name=f"s_{hc}_{b}_{jq}_{ck}")
                        ksl = k_sb[:, base + 128 * ck:base + 128 * (ck + 1)]
                        nc.tensor.matmul(
                            s_ps[:], ksl, qsl, start=True, stop=True
                        )
                        m = ck - 4 * jq
                        if m >= 0:
                            nc.vector.tensor_add(
                                s_ps[:], s_ps[:],
                                masks_sb[:, TQ * m:TQ * (m + 1)],
                            )
                        p_sb = pep.tile([128, TQ], BF16, tag="p",
                                        name=f"p_{hc}_{b}_{jq}_{ck}")
                        nc.scalar.activation(
                            p_sb[:], s_ps[:],
                            mybir.ActivationFunctionType.Exp,
                            scale=cfg.SCALE,
                        )
                        pr = p_sb[:]
                        g = (S // 128) * b + ck
                        vsl = v_sb[:, hc * T + 128 * g:hc * T + 128 * (g + 1)]
                        first, last = ck == 0, ck == nkc - 1
                        nc.tensor.matmul(
                            o_ps[:], vsl, pr, start=first, stop=last
                        )
                        nc.tensor.matmul(
                            l_ps[:], onesr, pr, start=first, stop=last
                        )
                    linv = lip.tile([128, TQ], F32, tag="li", name=f"li_{hc}_{b}_{jq}")
                    nc.vector.reciprocal_approx_fast(linv[:], l_ps[:])
                    nc.vector.tensor_mul(
                        oT_sb[:, base + TQ * jq:base + TQ * (jq + 1)],
                        o_ps[:], linv[:],
                    )


def _phase3_outproj(nc, tc, cfg, qkvp, oT_sb, wout_sb, out_d):
    T, S, HPC, DCH, TPC = cfg.T, cfg.S, cfg.HPC, cfg.DCH, cfg.TPC
    D, JW = cfg.D, cfg.JW

    with (
        tc.tile_pool(name="dram", bufs=1, space="DRAM") as dramp,
        tc.tile_pool(name="osb", bufs=3) as osbp,
        tc.tile_pool(name="pout", bufs=3, space="PSUM") as poutp,
    ):
        # One A2A per head so the first fires while the second head's
        # attention is still running (collectives run on CC cores/links,
        # not the 5 engines). Payload is bf16: 1MB per head per core.
        a2a_outs = []
        for hc in range(HPC):
            ob = dramp.tile([cfg.NC * 128, TPC], BF16, name=f"obounce{hc}")
            for s in range(cfg.NC):
                t0 = TPC * s
                b, sb0 = t0 // S, t0 % S
                nc.sync.dma_start(
                    ob[128 * s:128 * (s + 1), :],
                    oT_sb[:, hc * T + S * b + sb0:
                          hc * T + S * b + sb0 + TPC],
                )
            ao = dramp.tile([cfg.NC * 128, TPC], BF16, name=f"a2a_out{hc}")
            nc.gpsimd.collective_compute(
                "AllToAll",
                mybir.AluOpType.bypass,
                replica_groups=[list(range(cfg.NC))],
                ins=[ob[:].opt()],
                outs=[ao[:].opt()],
            )
            a2a_outs.append(ao)

        # Recycle the dead q/k/v SBUF slots (tag-shared): gathered o^T (bf16)
        # in v's slot, fp32 round-A partials in q's and k's slots.
        rhs_scr = qkvp.tile([128, DCH * TPC], BF16, tag="v_sb", name="rhs_scr")
        part_q = qkvp.tile([128, 8 * JW], F32, tag="q_sb", name="part_q")
        part_k = qkvp.tile([128, 8 * JW], F32, tag="k_sb", name="part_k")

        def part(js, ti):
            p = 4 * js + ti
            base = part_q if p < 8 else part_k
            return base[:, JW * (p % 8):JW * (p % 8 + 1)]

        rhs = [rhs_scr[:, TPC * fc:TPC * (fc + 1)] for fc in range(DCH)]
        for rnd in range(2):  # rnd 0: even heads (first A2A); 1: odd heads
            fcs = [fc for fc in range(DCH) if fc % HPC == rnd]
            for fc in fcs:
                nc.sync.dma_start(
                    rhs[fc], a2a_outs[rnd][128 * (fc // HPC):128 * (fc // HPC + 1), :]
                )
            for js in range(cfg.NJS):
                for ti in range(cfg.NTI):
                    ps = poutp.tile([128, JW], F32, tag="po",
                                    name=f"po_{rnd}_{js}_{ti}")
                    for i, fc in enumerate(fcs):
                        nc.tensor.matmul(
                            ps[:],
                            rhs[fc][:, 128 * ti:128 * (ti + 1)],
                            wout_sb[:, D * fc + JW * js:D * fc + JW * (js + 1)],
                            start=(i == 0), stop=(i == len(fcs) - 1),
                        )
                    if rnd == 0:
                        nc.vector.tensor_copy(part(js, ti), ps[:])
                    else:
                        osb = osbp.tile([128, JW], BF16, tag="ob",
                                        name=f"ob_{js}_{ti}")
                        nc.vector.tensor_add(osb[:], ps[:], part(js, ti))
                        nc.sync.dma_start(
                            out_d[128 * ti:128 * (ti + 1), JW * js:JW * (js + 1)],
                            osb[:],
                        )


def _build_body(nc, tc, cfg, xT_d, wqkvT_d, cosF_d, sinF_d, masks_d, ones_d, woutT_d, out_d):
    T, HPC, D, DCH = cfg.T, cfg.HPC, cfg.D, cfg.DCH

    with tc.tile_pool(name="const", bufs=1) as constp:
        ones_sb = constp.tile([128, 128], BF16)
        nc.sync.dma_start(ones_sb[:], ones_d[:])

        with (
            tc.tile_pool(name="woutp", bufs=1) as wop,
            tc.tile_pool(name="qkvp", bufs=1) as qkvp,
        ):
            # w_out stays resident in SBUF (bf16, 64KB/partition); its chunks
            # are streamed in during phase 1 (see _phase1_qkv_rope) so the
            # out-projection never waits on weight DMAs.
            wout_sb = wop.tile([128, DCH * D], BF16)

            q_sb = qkvp.tile([128, HPC * T], BF16)
            k_sb = qkvp.tile([128, HPC * T], BF16)
            v_sb = qkvp.tile([128, HPC * T], BF16)

            _phase1_qkv_rope(
                nc, tc, cfg, xT_d, wqkvT_d, cosF_d, sinF_d,
                woutT_d, wout_sb, q_sb, k_sb, v_sb
            )

            with tc.tile_pool(name="oT", bufs=1) as otp:
                oT_sb = otp.tile([128, HPC * T], BF16)

                _phase2_attention(
                    nc, tc, cfg, q_sb, k_sb, v_sb, oT_sb, masks_d, ones_sb
                )
                _phase3_outproj(nc, tc, cfg, qkvp, oT_sb, wout_sb, out_d)


# --------------------------------------------------------------------------
# host entry point
# --------------------------------------------------------------------------

_CACHE = {}


def _compiled(cfg):
    key = (cfg.B, cfg.S, cfg.D, cfg.H, cfg.NC)
    if key not in _CACHE:
        _CACHE[key] = build_program(cfg)
    return _CACHE[key]


def make_in_maps(cfg, inputs):
    shared, wqkvT = host_prep(
        cfg, inputs["x"], inputs["w_qkv"], inputs["w_out"],
        inputs["cos"], inputs["sin"],
    )
    return [{**shared, "wqkvT": wqkvT[c]} for c in range(cfg.NC)]


def assemble(cfg, results):
    out = np.concatenate([results[c]["out"] for c in range(cfg.NC)], axis=0)
    return out.reshape(cfg.B, cfg.S, cfg.D).astype(np.float32)


def kernel(x, w_qkv, w_out, cos, sin):
    cfg = FULL
    nc = _compiled(cfg)
    in_maps = make_in_maps(cfg, dict(x=x, w_qkv=w_qkv, w_out=w_out, cos=cos, sin=sin))
    res = bass_utils.run_bass_kernel_spmd(nc, in_maps, core_ids=list(range(cfg.NC)))
    return assemble(cfg, res.results)
